# revision 19
# baseline (speedup 1.0000x reference)
"""Causal self-attention Trainium2 Bass kernel.

Problem: B=4, T=2048, D=1024, H=16, head_dim=64.
Sharding: 8 cores = (batch b in 0..3) x (head-group g in 0..1, 8 heads each).
Each core computes a partial projection output for its batch over its 512
model dims; the host sums the two partials per batch (b_proj is fed to the
g==0 core only).

All matmul operands are float16 (PSUM accumulation stays fp32; biases and
the final output stay fp32).  The tensor engine streams moving rows at the
same rate for fp16 and fp32r, but fp16 halves SBUF/DMA and lets everything
double-buffer.  Attention processes queries in 1024-wide halves so both the
score PSUM and the y-accumulator PSUM fit double-buffered (2+2+2+2 banks),
which removes the per-head pipeline stalls.  The softmax denominator is
produced by 64 ones-columns appended to each head's v tile (rows 64..127 of
the y-accumulator), so normalization is a fast approx-reciprocal plus one
multiply with no cross-partition broadcast.
"""

import numpy as np

import concourse.bacc as bacc
import concourse.bass as bass
import concourse.mybir as mybir
import concourse.tile as tile
from concourse.bass_utils import run_bass_kernel_spmd

F32 = mybir.dt.float32
F16 = mybir.dt.float16
AF = mybir.ActivationFunctionType

B, T, D, H = 4, 2048, 1024, 16
HD = 64              # head dim
HPC = 8              # heads per core
DC = HPC * HD        # 512 model dims per core
SCALE = 1.0 / np.sqrt(HD)

# "exact": DVE reciprocal for the softmax denominator (no ACT table
# reloads); the final normalization uses the scalar-engine ln/exp path so
# phase C's PSUM banks free ~3us earlier.  ("lnexp" everywhere is slower:
# each Exp<->Ln switch reloads the ACT table, ~1.3us a pop.
# reciprocal_approx_fast is broken on this hardware - returns garbage.)
RECIP_MODE = "exact"

_NC_CACHE = {}


def build_nc(t=T, reps=1):
    """Build the single-core SPMD program. t = sequence length (for small
    sims). reps>1 repeats the computation (for repeat-delta timing)."""
    nt = t // 128          # 128-row tiles over time
    nq = t // 512          # 512-col chunks over time
    KC = D // 128          # 8 contraction chunks for qkv
    MQK = DC // 128        # 4 feature tiles for each of q,k

    nc = bacc.Bacc("TRN2", target_bir_lowering=False, debug=False)

    xT_d = nc.dram_tensor("xT", [D, t], F16, kind="ExternalInput")
    wq_d = nc.dram_tensor("wq", [D, DC], F16, kind="ExternalInput")
    wk_d = nc.dram_tensor("wk", [D, DC], F16, kind="ExternalInput")
    wv_d = nc.dram_tensor("wv", [D, DC], F16, kind="ExternalInput")
    bq_d = nc.dram_tensor("bq", [1, DC], F32, kind="ExternalInput")
    bk_d = nc.dram_tensor("bk", [1, DC], F32, kind="ExternalInput")
    bv_d = nc.dram_tensor("bv", [1, DC], F32, kind="ExternalInput")
    wp_d = nc.dram_tensor("wp", [DC, D], F16, kind="ExternalInput")
    bp_d = nc.dram_tensor("bp", [1, D], F32, kind="ExternalInput")
    out_d = nc.dram_tensor("out", [t, D], F32, kind="ExternalOutput")

    with tile.TileContext(nc) as tc:
      for _rep in range(reps):
        with tc.tile_pool(name="persist", bufs=1) as persist, \
             tc.tile_pool(name="vpool", bufs=1) as vpool, \
             tc.tile_pool(name="qkpool", bufs=1) as qkpool:

            # resident qk^T: [:, m, :] = q^T feats tile m, [:, 4+m, :] = k^T
            qkTb = qkpool.tile([128, 2 * MQK, t], F16)

            # broadcast bias constants
            bv_bc = persist.tile([128, DC], F32)
            nc.gpsimd.dma_start(bv_bc[:], bv_d[0:1, :].to_broadcast([128, DC]))
            bp_bc = persist.tile([128, D], F32)
            nc.gpsimd.dma_start(bp_bc[:], bp_d[0:1, :].to_broadcast([128, D]))
            # partition-major per-feature-tile bias columns [128, MQK]
            bqp = persist.tile([128, MQK], F32)
            nc.sync.dma_start(bqp[:], bq_d.rearrange("o (m p) -> p (o m)", p=128))
            bkp = persist.tile([128, MQK], F32)
            nc.sync.dma_start(bkp[:], bk_d.rearrange("o (m p) -> p (o m)", p=128))

            # v' mega-tile: [128, nt, 8*128]; per head h, cols h*128+64 ..
            # h*128+127 hold ones so the av matmul also produces the softmax
            # denominator replicated on PSUM rows 64..127.  Only the ones
            # columns are memset; the v evacuations fill cols 0..63.
            vpm = vpool.tile([128, nt, HPC * 2 * HD], F16)
            nc.gpsimd.memset(
                vpm.rearrange("p t (h e) -> p (t h) e", e=2 * HD)[:, :, HD:2 * HD],
                1.0)

            # ---------------- Phase A: qkv ----------------
            with tc.tile_pool(name="phA_sb", bufs=1) as pa, \
                 tc.tile_pool(name="phA_w", bufs=2) as pw:

                # v weights first (small, unblocks the first matmuls), then
                # x^T in two halves on the sync queue so compute starts early
                wvs = pw.tile([128, KC, DC], F16, name="wvs", tag="wsec")
                nc.scalar.dma_start(wvs[:], wv_d.rearrange("(k p) c -> p k c", p=128))
                # t-major quarters: the first v tiles need all k-chunks but
                # only the first t columns, so compute starts after ~1MB
                xTb = pa.tile([128, KC, t], F16)
                for tq in range(0, t, 512):
                    nc.sync.dma_start(
                        xTb[:, :, tq:tq + 512],
                        xT_d[:, tq:tq + 512].rearrange("(k p) t -> p k t", p=128))
                with tc.tile_pool(name="phA_vps", bufs=4, space="PSUM") as pvps:
                    for tt in range(nt):
                        ps = pvps.tile([128, DC], F32, name="vps", tag="psv")
                        for k in range(KC):
                            nc.tensor.matmul(
                                ps[:],
                                xTb[:, k, tt * 128:(tt + 1) * 128],
                                wvs[:, k, :],
                                start=(k == 0), stop=(k == KC - 1))
                        nc.vector.tensor_add(
                            vpm[:, tt].rearrange("p (h e) -> p h e", e=2 * HD)[:, :, 0:HD],
                            ps.rearrange("p (h e) -> p h e", e=HD),
                            bv_bc.rearrange("p (h e) -> p h e", e=HD))

                # q^T / k^T -> psum -> (bias-add) resident qkTb; weight-
                # stationary loop order (stationary fixed across n)
                with tc.tile_pool(name="phA_qkps", bufs=2, space="PSUM") as pps:
                    for sec, (w_d, b_s) in enumerate(((wq_d, bqp), (wk_d, bkp))):
                        ws = pw.tile([128, KC, DC], F16, name=f"ws{sec}", tag="wsec")
                        nc.sync.dma_start(ws[:], w_d.rearrange("(k p) c -> p k c", p=128))
                        for m in range(MQK):
                            ps = pps.tile([128, t], F32, name="qkps", tag="psqk")
                            for k in range(KC):
                                for n in range(nq):
                                    nc.tensor.matmul(
                                        ps[:, n * 512:(n + 1) * 512],
                                        ws[:, k, m * 128:(m + 1) * 128],
                                        xTb[:, k, n * 512:(n + 1) * 512],
                                        start=(k == 0), stop=(k == KC - 1))
                            nc.vector.tensor_scalar_add(
                                qkTb[:, sec * MQK + m, :], ps[:], b_s[:, m:m + 1])

            # ---------------- Phase B: attention ----------------
            # queries processed in 1024-wide halves so score PSUM and the
            # y-accumulator both double-buffer: (2+2)+(2+2) = 8 banks.
            with tc.tile_pool(name="yT", bufs=1) as ypool, \
                 tc.tile_pool(name="phC_sb", bufs=1) as pc:
                yT = [ypool.tile([128, t], F16, name=f"yT{f}", tag=f"yT{f}")
                      for f in range(MQK)]
                # prefetch projection weights during attention
                wpb = pc.tile([128, MQK, D], F16)
                nc.scalar.dma_start(
                    wpb[:], wp_d.rearrange("(m p) o -> p m o", p=128))

                with tc.tile_pool(name="esb", bufs=3) as pesb, \
                     tc.tile_pool(name="rec", bufs=2) as prec, \
                     tc.tile_pool(name="sc_ps", bufs=2, space="PSUM") as pscps, \
                     tc.tile_pool(name="y_ps", bufs=2, space="PSUM") as pyps:

                    nqg = t // 1024        # query groups
                    for f in range(MQK):
                        for hh in range(2):
                            h = 2 * f + hh
                            qh = qkTb[:, f][hh * HD:(hh + 1) * HD, :]
                            kh = qkTb[:, MQK + f][hh * HD:(hh + 1) * HD, :]
                            for qg in range(nqg):
                                q0 = qg * 1024
                                y_acc = pyps.tile([128, 1024], F32,
                                                  name="yacc", tag="yacc")
                                # keys 0 .. (end of this query group);
                                # av emission trails scores by one kc so the
                                # tensor queue never waits on the exp
                                pending = None

                                def emit_av(kc, spans, esb):
                                    for (a, b2) in spans:
                                        n = (q0 + a) // 512
                                        nc.tensor.matmul(
                                            y_acc[:, a:b2],
                                            vpm[:, kc, h * 128:(h + 1) * 128],
                                            esb[:, a:b2],
                                            start=(kc == 0),
                                            stop=(kc == 4 * n + 3))

                                for kc in range((qg + 1) * 8):
                                    dn = kc // 4      # diagonal 512-chunk
                                    dlo = kc * 128    # first live column
                                    sp = pscps.tile([128, 1024], F32,
                                                    name="scps", tag="scps")
                                    spans = []
                                    for n in (2 * qg, 2 * qg + 1):
                                        if n < dn:
                                            continue
                                        w0 = max(dlo, n * 512)
                                        spans.append((w0 - q0, (n + 1) * 512 - q0))
                                        nc.tensor.matmul(
                                            sp[:, w0 - q0:(n + 1) * 512 - q0],
                                            kh[:, kc * 128:(kc + 1) * 128],
                                            qh[:, w0:(n + 1) * 512],
                                            start=True, stop=True)
                                    r0 = spans[0][0]
                                    esb = pesb.tile([128, 1024], F16,
                                                    name="esb", tag="esb")
                                    nc.scalar.activation(
                                        esb[:, r0:], sp[:, r0:], AF.Exp,
                                        scale=float(SCALE))
                                    if dn >= 2 * qg:
                                        # zero the upper triangle of the
                                        # diagonal 128-col block (query u <
                                        # key p) on the idle gpsimd engine
                                        nc.gpsimd.affine_select(
                                            out=esb[:, r0:r0 + 128],
                                            in_=esb[:, r0:r0 + 128],
                                            compare_op=mybir.AluOpType.is_ge,
                                            fill=0.0, base=0,
                                            pattern=[[1, 128]],
                                            channel_multiplier=-1)
                                    if pending is not None:
                                        emit_av(*pending)
                                    pending = (kc, spans, esb)
                                emit_av(*pending)
                                # normalize: yT = y / denom (denom is on
                                # PSUM rows 64..127, replicated)
                                rec = prec.tile([64, 1024], F32,
                                                name="rec", tag="rec")
                                last = (f == MQK - 1 and hh == 1
                                        and qg == nqg - 1)
                                if RECIP_MODE == "exact" and not last:
                                    nc.vector.reciprocal(rec[:], y_acc[64:128, :])
                                elif RECIP_MODE == "lnexp" or last:
                                    # 1/d = exp(-ln d): two scalar-engine LUT
                                    # ops; keeps the DVE free
                                    lnd = prec.tile([64, 1024], F32,
                                                    name="lnd", tag="lnd")
                                    nc.scalar.activation(lnd[:], y_acc[64:128, :],
                                                         AF.Ln)
                                    nc.scalar.activation(rec[:], lnd[:], AF.Exp,
                                                         scale=-1.0)
                                else:
                                    nc.vector.reciprocal_approx_fast(
                                        rec[:], y_acc[64:128, :])
                                nc.vector.tensor_mul(
                                    yT[f][hh * HD:(hh + 1) * HD, q0:q0 + 1024],
                                    y_acc[0:HD, :], rec[:])

                # ---------------- Phase C: projection ----------------
                # 3 query-tiles per accumulation round, m-major, so the
                # first m=3 matmul comes ~8us in and the last head's
                # normalization finishes in the shadow.
                with tc.tile_pool(name="phC_evac", bufs=3) as pcev, \
                     tc.tile_pool(name="phC_ps", bufs=3, space="PSUM") as pcps:
                    for blk in range(0, nt, 3):
                        qts = list(range(blk, min(blk + 3, nt)))
                        pss = [pcps.tile([128, D], F32, name="prps", tag="prps")
                               for _ in qts]
                        for m in range(MQK):
                            for j, qt in enumerate(qts):
                                for oc in range(D // 512):
                                    nc.tensor.matmul(
                                        pss[j][:, oc * 512:(oc + 1) * 512],
                                        yT[m][:, qt * 128:(qt + 1) * 128],
                                        wpb[:, m, oc * 512:(oc + 1) * 512],
                                        start=(m == 0), stop=(m == MQK - 1))
                        for j, qt in enumerate(qts):
                            ev = pcev.tile([128, D], F32, name="prev", tag="prev")
                            nc.vector.tensor_add(ev[:], pss[j][:], bp_bc[:])
                            nc.sync.dma_start(
                                out_d[qt * 128:(qt + 1) * 128, :], ev[:])

    nc.finalize()
    return nc


def make_in_maps(x, w_attn, b_attn, w_proj, b_proj):
    x = np.asarray(x, dtype=np.float32)
    w_attn = np.asarray(w_attn, dtype=np.float32)
    b_attn = np.asarray(b_attn, dtype=np.float32)
    w_proj = np.asarray(w_proj, dtype=np.float32)
    b_proj = np.asarray(b_proj, dtype=np.float32)
    f16 = np.float16
    in_maps = []
    for c in range(8):
        b, g = c // 2, c % 2
        sl = slice(DC * g, DC * (g + 1))
        in_maps.append({
            "xT": np.ascontiguousarray(x[b].T.astype(f16)),
            "wq": np.ascontiguousarray(w_attn[:, 0 * D:][:, sl].astype(f16)),
            "wk": np.ascontiguousarray(w_attn[:, 1 * D:][:, sl].astype(f16)),
            "wv": np.ascontiguousarray(w_attn[:, 2 * D:][:, sl].astype(f16)),
            "bq": np.ascontiguousarray(b_attn[0 * D:1 * D][sl][None, :]),
            "bk": np.ascontiguousarray(b_attn[1 * D:2 * D][sl][None, :]),
            "bv": np.ascontiguousarray(b_attn[2 * D:3 * D][sl][None, :]),
            "wp": np.ascontiguousarray(w_proj[sl, :].astype(f16)),
            "bp": np.ascontiguousarray(
                (b_proj if g == 0 else np.zeros_like(b_proj))[None, :]),
        })
    return in_maps


def kernel(x, w_attn, b_attn, w_proj, b_proj):
    if "nc" not in _NC_CACHE:
        _NC_CACHE["nc"] = build_nc()
    nc = _NC_CACHE["nc"]
    in_maps = make_in_maps(x, w_attn, b_attn, w_proj, b_proj)
    res = run_bass_kernel_spmd(nc, in_maps, core_ids=list(range(8)))
    outs = [res.results[c]["out"] for c in range(8)]
    out = np.empty((B, T, D), dtype=np.float32)
    for b in range(B):
        np.add(outs[2 * b], outs[2 * b + 1], out=out[b])
    kernel._last_results = res
    return out


if __name__ == "__main__":
    nc = build_nc()
    print("built ok")
